# revision 1
# baseline (speedup 1.0000x reference)
"""ExpertsChooseMlp Trainium2 kernel.

Full inputs in, full output out. Sharding: 8 cores = 4 batches x 2 expert-pairs.
Core m handles batch b=m//2 and experts {2g, 2g+1}, g=m%2. Each core computes
pout[T,O] = sum_{e in pair} combine[b,:,e,:] @ mlp_e(dispatch[b,:,e,:]^T @ x[b]);
the host sums the two partials per batch and adds b2.

Precision: x/w1/w2/h in bf16, dispatch_mask/combine_array/y in fp8-e4m3, all
accumulation in fp32 PSUM (measured 4e-3 max relative error vs the fp32
reference). The combine contraction runs as fp8 DoubleRow matmuls (2 C-planes
per pass, ~1.8x bf16 throughput). Layouts are chosen so the natural
(host-prepared) operand orientations feed the PE with zero on-device
transposes:
  xdT[D,C] = matmul(lhsT=x[b][T,D],  rhs=dm_e[T,C])     (K=T)
  hT[HE,C] = matmul(lhsT=w1_e[D,HE], rhs=xdT[D,C])      (K=D), then GELU+b1
  y[C,O]   = matmul(lhsT=hT[HE,C],   rhs=w2_e[HE,O])    (K=HE)
  pout[T,O]= matmul(lhsT=cmT_e[C,T], rhs=y[C,O])        (K=C, accum over e,
                                                         fp8 DoubleRow)
Only cmT (combine slice transposed) is materialized host-side.
Measured: ~128us HW exec per core (all 8 cores balanced, PE dense with zero
>200ns gaps between matmuls; remaining overhead is engine preamble + Tile
exit barrier).
"""
import sys

sys.path.insert(0, "/opt/trn_rl_repo")

import numpy as np
import ml_dtypes

import concourse.bacc as bacc
import concourse.mybir as mybir
import concourse.tile as tile
from concourse import bass_utils

B, T, D, E, C, HE, O = 4, 2048, 512, 4, 1024, 512, 512
P = 128
nKT = T // P      # 16 T-chunks
nMD = D // P      # 4  D-chunks
nMH = HE // P     # 4  HE-chunks
nKD = D // P      # 4
nCC = C // P      # 8  C-chunks
nKH = HE // P     # 4
nMT = T // P      # 16
NF = 512          # matmul free dim (one PSUM bank)

F32 = mybir.dt.float32
BF16 = mybir.dt.bfloat16
F8 = mybir.dt.float8e4
GELU = mybir.ActivationFunctionType.Gelu
DR = mybir.MatmulPerfMode.DoubleRow
nCP = nCC // 2    # 4  C pair-chunks (DoubleRow: K=256 per matmul)

_NC = None


def _build():
    nc = bacc.Bacc("TRN2", target_bir_lowering=False, debug=False,
                   enable_asserts=False, num_devices=1)
    xb = nc.dram_tensor("xb", [T, D], BF16, kind="ExternalInput").ap()
    dm = nc.dram_tensor("dm", [2, T, C], F8, kind="ExternalInput").ap()
    cmt = nc.dram_tensor("cmt", [2, C, T], F8, kind="ExternalInput").ap()
    w1 = nc.dram_tensor("w1", [2, D, HE], BF16, kind="ExternalInput").ap()
    w2 = nc.dram_tensor("w2", [2, HE, O], BF16, kind="ExternalInput").ap()
    b1 = nc.dram_tensor("b1s", [2, HE], F32, kind="ExternalInput").ap()
    pout = nc.dram_tensor("pout", [T, O], F32, kind="ExternalOutput").ap()

    with tile.TileContext(nc) as tc:
        with (
            tc.tile_pool(name="const", bufs=1) as const,
            tc.tile_pool(name="dmp", bufs=32) as dmp,
            tc.tile_pool(name="cmp", bufs=8) as cmp_,
            tc.tile_pool(name="inter", bufs=1) as inter,
            tc.tile_pool(name="yp", bufs=2) as yp,
            tc.tile_pool(name="outp", bufs=2) as outp,
            tc.tile_pool(name="psum", bufs=8, space="PSUM") as psp,
        ):
            # ---- resident constants (ACT HWDGE ring) ----
            # x split per-chunk so the first matmul isn't gated on a 2MB DMA
            x_sb = const.tile([P, nKT, D], BF16)
            for kt in range(nKT):
                nc.scalar.dma_start(x_sb[:, kt, :], xb[kt * P:(kt + 1) * P, :])
            w1_sb = const.tile([P, 2, nKD, HE], BF16)
            nc.scalar.dma_start(w1_sb[:], w1.rearrange("e (kd p) j -> p e kd j", p=P))
            w2_sb = const.tile([P, 2, nKH, O], BF16)
            nc.scalar.dma_start(w2_sb[:], w2.rearrange("e (kh p) j -> p e kh j", p=P))
            b1_sb = const.tile([P, 2 * nMH], F32)
            nc.scalar.dma_start(b1_sb[:], b1.rearrange("e (mh p) -> p (e mh)", p=P))

            # ---- HAM warmup: ~4us of dummy matmuls on uninitialized SBUF
            # during the initial DMA wait, so real matmuls start at 2.4GHz.
            # Results go to a scratch PSUM bank and are discarded.
            warm = const.tile([P, NF], BF16)
            nc.gpsimd.memset(warm[:], 0.0)
            ps_w = psp.tile([P, NF], F32, tag="ps", name="ps_warm")
            for i in range(8):
                nc.tensor.matmul(ps_w[:], warm[:, 0:P], warm[:],
                                 start=(i == 0), stop=(i == 7))

            y_tiles = []
            for ei in range(2):
                # ---- dispatch-mask tiles for this expert (SYNC ring) ----
                dm_t = []
                for kt in range(nKT):
                    t_ = dmp.tile([P, C], F8, tag="dm")
                    nc.sync.dma_start(t_[:], dm[ei, kt * P:(kt + 1) * P, :])
                    dm_t.append(t_)

                # ---- phase A: xdT[D, C] ----
                # kt-outer: all 8 PSUM banks accumulate in parallel, so each
                # dm tile is consumed once (at sustainable DMA rate) and
                # released immediately for the next expert's prefetch.
                xdt = inter.tile([P, nMD, C], BF16, tag="xdt")
                pss = [psp.tile([P, NF], F32, tag="ps", name=f"psa{i}")
                       for i in range(2 * nMD)]
                for kt in range(nKT):
                    for mc in range(nMD):
                        lhsT = x_sb[:, kt, mc * P:(mc + 1) * P]
                        nc.tensor.matmul(pss[2 * mc][:], lhsT, dm_t[kt][:, 0:NF],
                                         start=(kt == 0), stop=(kt == nKT - 1))
                        nc.tensor.matmul(pss[2 * mc + 1][:], lhsT, dm_t[kt][:, NF:C],
                                         start=(kt == 0), stop=(kt == nKT - 1))
                for ncc in range(2):
                    for mc in range(nMD):
                        nc.vector.tensor_copy(xdt[:, mc, ncc * NF:(ncc + 1) * NF],
                                              pss[2 * mc + ncc][:])

                # ---- phase B: hT[HE, C] = gelu(w1^T xdT + b1) ----
                # ncc-outer so phase C's first C-half unblocks after 4 gelus.
                # (kd-outer over 8 PSUM banks measured WORSE here: holding all
                # banks serializes the A->B transition.)
                ht = inter.tile([P, nMH, C], BF16, tag="ht")
                for ncc in range(2):
                    sl = slice(ncc * NF, (ncc + 1) * NF)
                    for mh in range(nMH):
                        ps0 = psp.tile([P, NF], F32, tag="ps")
                        for kd in range(nKD):
                            nc.tensor.matmul(ps0[:],
                                             w1_sb[:, ei, kd, mh * P:(mh + 1) * P],
                                             xdt[:, kd, sl],
                                             start=(kd == 0), stop=(kd == nKD - 1))
                        bia = b1_sb[:, ei * nMH + mh:ei * nMH + mh + 1]
                        nc.scalar.activation(ht[:, mh, sl], ps0[:], GELU, bias=bia)

                # ---- phase C: y[C, O] (stored fp8, DoubleRow plane layout:
                # row c = kp*256 + i*128 + p  ->  y_sb[p, kp, i, :]) ----
                y_sb = yp.tile([P, nCP, 2, O], F8, tag="y")
                for cc in range(nCC):
                    ps = psp.tile([P, NF], F32, tag="ps")
                    for kh in range(nKH):
                        nc.tensor.matmul(ps[:], ht[:, kh, cc * P:(cc + 1) * P],
                                         w2_sb[:, ei, kh, :],
                                         start=(kh == 0), stop=(kh == nKH - 1))
                    nc.vector.tensor_copy(y_sb[:, cc // 2, cc % 2, :], ps[:])
                y_tiles.append(y_sb)

            # ---- combine-mask tiles (fp8, [P, plane, T]): SYNC ring behind
            # the dm loads so they can't steal early HBM bandwidth ----
            cmt_t = {}
            for ei in range(2):
                for kp in range(nCP):
                    t_ = cmp_.tile([P, 2, T], F8, tag="cmt")
                    nc.sync.dma_start(
                        t_[:],
                        cmt[ei, kp * 2 * P:(kp + 1) * 2 * P, :]
                        .rearrange("(i p) t -> p i t", p=P))
                    cmt_t[(ei, kp)] = t_

            # ---- phase D: pout[T, O] = sum_e cmT_e^T y_e (fp8 DoubleRow) ----
            for mt in range(nMT):
                ps = psp.tile([P, NF], F32, tag="ps")
                idx = 0
                for ei in range(2):
                    for kp in range(nCP):
                        nc.tensor.matmul(ps[:],
                                         cmt_t[(ei, kp)][:, :, mt * P:(mt + 1) * P],
                                         y_tiles[ei][:, kp, :, :],
                                         start=(idx == 0), stop=(idx == 7),
                                         perf_mode=DR)
                        idx += 1
                ot = outp.tile([P, O], F32, tag="out")
                nc.vector.tensor_copy(ot[:], ps[:])
                nc.sync.dma_start(pout[mt * P:(mt + 1) * P, :], ot[:])

    nc.compile()
    return nc


def get_nc():
    global _NC
    if _NC is None:
        _NC = _build()
    return _NC


def make_in_maps(x, dispatch_mask, combine_array, w1, b1, w2):
    bf = ml_dtypes.bfloat16
    in_maps = []
    for m in range(8):
        b, g = m // 2, m % 2
        es = slice(2 * g, 2 * g + 2)
        dm_s = np.ascontiguousarray(
            np.transpose(dispatch_mask[b, :, es, :], (1, 0, 2))).astype(
                ml_dtypes.float8_e4m3)
        cmt_s = np.ascontiguousarray(
            np.transpose(combine_array[b, :, es, :], (1, 2, 0))).astype(
                ml_dtypes.float8_e4m3)
        in_maps.append({
            "xb": np.ascontiguousarray(x[b]).astype(bf),
            "dm": dm_s,
            "cmt": cmt_s,
            "w1": np.ascontiguousarray(w1[es]).astype(bf),
            "w2": np.ascontiguousarray(w2[es]).astype(bf),
            "b1s": np.ascontiguousarray(b1[es]).astype(np.float32),
        })
    return in_maps


def kernel(x, dispatch_mask, combine_array, w1, b1, w2, b2):
    nc = get_nc()
    x, dispatch_mask, combine_array, w1, b1, w2 = (
        np.asarray(a, dtype=np.float32)
        for a in (x, dispatch_mask, combine_array, w1, b1, w2))
    in_maps = make_in_maps(x, dispatch_mask, combine_array, w1, b1, w2)
    res = bass_utils.run_bass_kernel_spmd(nc, in_maps, core_ids=list(range(8)))
    b2f = np.asarray(b2, dtype=np.float32)
    out = np.empty((B, T, O), dtype=np.float32)
    for b in range(B):
        out[b] = res.results[2 * b]["pout"] + res.results[2 * b + 1]["pout"] + b2f
    return out



# revision 8
# speedup vs baseline: 1.4342x; 1.4342x over previous
"""ExpertsChooseMlp Trainium2 kernel.

Full inputs in, full output out. Sharding: 8 cores = 4 batches x 2 expert-pairs.
Core m handles batch b=m//2 and experts {2g, 2g+1}, g=m%2. Each core computes
pout[T,O] = sum_{e in pair} combine[b,:,e,:] @ mlp_e(dispatch[b,:,e,:]^T @ x[b]);
the host sums the two partials per batch, applies the w2 rank-1 correction and
adds b2.

All four contractions run as fp8-e4m3 DoubleRow matmuls (2 K-planes of 128 per
pass, ~1.97x bf16 throughput at FD=512, LDWEIGHTS fully hidden), fp32 PSUM
accumulation.

Precision design (the output is dominated by the capacity-mean channel
0.5*sum_c y, so any quantization error that is coherent across the capacity
dim passes straight through at ~2.5% while incoherent error averages away
~30x; each coherent channel is therefore computed exactly on the host):
  - dispatch runs on v8 = fp8(dm - 0.5) (zero-mean), and the removed mean
    term 0.5 * w1_true^T colsum(x_true) is folded (fp64, host) into the
    per-partition GELU bias -> kills the x and w1 coherent channels.
  - w2's coherent channel is removed by a host rank-1 correction
    rowsum(cm8) (x) (ghsum @ (w2q/16 - w2))/C, where ghsum = sum_c gelu
    output is measured exactly on-device via activation accum_out (free).
  - w1/w2 are pre-scaled by 16 so their fp8 values avoid subnormals; the
    1/16 unscale is folded into the ScalarE activations (GELU / Copy).
Emulated end-to-end: rel ~ 6.6e-3 (gate 2e-2).

All DMA operands are pre-arranged on the host so device DMAs move >=1KB
contiguous per partition (no on-device rearranges; descriptor-rate limited
DMAs were gating the first matmul). No warmup block: HAM ramps on the first
real phase-A matmuls (~8us).
"""
import sys

sys.path.insert(0, "/opt/trn_rl_repo")

import numpy as np
import ml_dtypes

import concourse.bacc as bacc
import concourse.mybir as mybir
import concourse.tile as tile
from concourse import bass_utils

B, T, D, E, C, HE, O = 4, 2048, 512, 4, 1024, 512, 512
P = 128
nKP = T // (2 * P)  # 8  T pair-chunks (phase A DoubleRow: K=256 per matmul)
nMD = D // P        # 4  D-chunks
nMH = HE // P       # 4  HE-chunks
nKDP = D // (2 * P)   # 2  D pair-chunks (phase B DR)
nKHP = HE // (2 * P)  # 2  HE pair-chunks (phase C DR)
nCC = C // P        # 8  C-chunks
nMT = T // P        # 16
NF = 512            # matmul free dim (one PSUM bank)
WS = 16.0           # host-side w1/w2 scale (keeps fp8 weights out of subnormals)

F32 = mybir.dt.float32
F8 = mybir.dt.float8e4
GELU = mybir.ActivationFunctionType.Gelu
COPY = mybir.ActivationFunctionType.Copy
DR = mybir.MatmulPerfMode.DoubleRow
nCP = nCC // 2      # 4  C pair-chunks (phase D DR)

_NC = None


def _build():
    nc = bacc.Bacc("TRN2", target_bir_lowering=False, debug=False,
                   enable_asserts=False, num_devices=1)
    # host-prearranged layouts: partition dim is explicit so every DMA is
    # contiguous per partition
    xb = nc.dram_tensor("xb", [P, nKP, 2, D], F8, kind="ExternalInput").ap()
    dm = nc.dram_tensor("dm", [2, nKP, P, 2, C], F8, kind="ExternalInput").ap()
    cmt = nc.dram_tensor("cmt", [2, nCP, P, 2, T], F8, kind="ExternalInput").ap()
    w1 = nc.dram_tensor("w1", [P, 2, nKDP, 2, HE], F8, kind="ExternalInput").ap()
    w2 = nc.dram_tensor("w2", [P, 2, nKHP, 2, O], F8, kind="ExternalInput").ap()
    b1 = nc.dram_tensor("b1s", [P, 2 * nMH], F32, kind="ExternalInput").ap()
    pout = nc.dram_tensor("pout", [T, O], F32, kind="ExternalOutput").ap()
    ghs = nc.dram_tensor("ghs", [P, 2, 2, nMH], F32, kind="ExternalOutput").ap()

    with tile.TileContext(nc) as tc:
        with (
            tc.tile_pool(name="const", bufs=1) as const,
            tc.tile_pool(name="dmp", bufs=16) as dmp,
            tc.tile_pool(name="cmp", bufs=8) as cmp_,
            tc.tile_pool(name="inter", bufs=1) as inter,
            tc.tile_pool(name="yp", bufs=2) as yp,
            tc.tile_pool(name="outp", bufs=6) as outp,
            tc.tile_pool(name="gscp", bufs=2) as gscp,
            tc.tile_pool(name="psum", bufs=8, space="PSUM") as psp,
        ):
            # ---- resident constants ----
            # x split per pair-chunk so the first matmul isn't gated on a
            # full-tensor DMA (ACT HWDGE ring, ahead of the weights)
            x_sb = const.tile([P, nKP, 2, D], F8)
            for kp in range(nKP):
                nc.scalar.dma_start(x_sb[:, kp, :, :], xb[:, kp, :, :])
            w1_sb = const.tile([P, 2, nKDP, 2, HE], F8)
            nc.scalar.dma_start(w1_sb[:], w1[:])
            w2_sb = const.tile([P, 2, nKHP, 2, O], F8)
            nc.scalar.dma_start(w2_sb[:], w2[:])
            b1_sb = const.tile([P, 2 * nMH], F32)
            nc.scalar.dma_start(b1_sb[:], b1[:])
            acc = const.tile([P, 2, 2, nMH], F32)

            y_tiles = []
            for ei in range(2):
                # ---- dispatch-mask pair-tiles for this expert (SYNC ring) ----
                dm_t = []
                for kp in range(nKP):
                    t_ = dmp.tile([P, 2, C], F8, tag="dm")
                    nc.sync.dma_start(t_[:], dm[ei, kp])
                    dm_t.append(t_)

                # ---- phase A: xdT[D, C] = x^T (dm-0.5)  (fp8 DR, K=T) ----
                # kp-outer: all 8 PSUM banks accumulate in parallel, so each
                # dm tile is consumed once (at sustainable DMA rate) and
                # released immediately for the next expert's prefetch. On the
                # last kp the half=0 banks stop first so phase B's operands
                # drain to SBUF before phase A's matmuls finish.
                xdt = inter.tile([P, nMD, C], F8, tag="xdt")
                pss = [psp.tile([P, NF], F32, tag="ps", name=f"psa{i}")
                       for i in range(2 * nMD)]
                for kp in range(nKP):
                    if kp == nKP - 1:
                        order = [(mc, h) for h in range(2) for mc in range(nMD)]
                    else:
                        order = [(mc, h) for mc in range(nMD) for h in range(2)]
                    for mc, h in order:
                        nc.tensor.matmul(pss[2 * mc + h][:],
                                         x_sb[:, kp, :, mc * P:(mc + 1) * P],
                                         dm_t[kp][:, :, h * NF:(h + 1) * NF],
                                         start=(kp == 0), stop=(kp == nKP - 1),
                                         perf_mode=DR)
                for ncc in range(2):
                    for mc in range(nMD):
                        nc.vector.tensor_copy(xdt[:, mc, ncc * NF:(ncc + 1) * NF],
                                              pss[2 * mc + ncc][:])

                # ---- phase B: hT[HE, C] = gelu(w1^T xdT / WS + bias) (DR) ----
                # ncc-outer so phase C's first C-half unblocks after 4 gelus;
                # kp-outer within so the first matmuls only need the first
                # two xdt planes. accum_out captures sum_c gelu exactly for
                # the host-side w2 correction.
                ht = inter.tile([P, nMH, C], F8, tag="ht")
                for ncc in range(2):
                    sl = slice(ncc * NF, (ncc + 1) * NF)
                    bss = [psp.tile([P, NF], F32, tag="ps", name=f"psb{i}")
                           for i in range(nMH)]
                    for kp in range(nKDP):
                        for mh in range(nMH):
                            nc.tensor.matmul(
                                bss[mh][:],
                                w1_sb[:, ei, kp, :, mh * P:(mh + 1) * P],
                                xdt[:, 2 * kp:2 * kp + 2, sl],
                                start=(kp == 0), stop=(kp == nKDP - 1),
                                perf_mode=DR)
                    for mh in range(nMH):
                        bia = b1_sb[:, ei * nMH + mh:ei * nMH + mh + 1]
                        nc.scalar.activation(ht[:, mh, sl], bss[mh][:], GELU,
                                             bias=bia, scale=1.0 / WS)

                # ---- ghsum: sum_c of the fp8 gelu values via DVE pass-through
                # with accum_out (feeds the host-side w2 rank-1 correction) ----
                for ncc in range(2):
                    sl = slice(ncc * NF, (ncc + 1) * NF)
                    for mh in range(nMH):
                        gsc = gscp.tile([P, NF], F8, tag="gsc")
                        nc.vector.tensor_scalar(
                            gsc[:], ht[:, mh, sl], 0.0, None,
                            mybir.AluOpType.add, mybir.AluOpType.add,
                            accum_out=acc[:, ei, ncc, mh:mh + 1])

                # ---- phase C: y[C, O] (fp8 DR; stored fp8, DR plane layout:
                # row c = kp*256 + i*128 + p  ->  y_sb[p, kp, i, :]) ----
                y_sb = yp.tile([P, nCP, 2, O], F8, tag="y")
                for cc in range(nCC):
                    ps = psp.tile([P, NF], F32, tag="ps")
                    for kp in range(nKHP):
                        nc.tensor.matmul(ps[:],
                                         ht[:, 2 * kp:2 * kp + 2, cc * P:(cc + 1) * P],
                                         w2_sb[:, ei, kp, :, :],
                                         start=(kp == 0), stop=(kp == nKHP - 1),
                                         perf_mode=DR)
                    nc.scalar.activation(y_sb[:, cc // 2, cc % 2, :], ps[:],
                                         COPY, scale=1.0 / WS)
                y_tiles.append(y_sb)

            nc.sync.dma_start(ghs[:], acc[:])

            # ---- combine-mask tiles (fp8, [P, plane, T]): SYNC ring behind
            # the dm loads so they can't steal early HBM bandwidth ----
            cmt_t = {}
            for ei in range(2):
                for kp in range(nCP):
                    t_ = cmp_.tile([P, 2, T], F8, tag="cmt")
                    nc.sync.dma_start(t_[:], cmt[ei, kp])
                    cmt_t[(ei, kp)] = t_

            # ---- phase D: pout[T, O] = sum_e cmT_e^T y_e (fp8 DR) ----
            for mt in range(nMT):
                ps = psp.tile([P, NF], F32, tag="ps")
                idx = 0
                for ei in range(2):
                    for kp in range(nCP):
                        nc.tensor.matmul(ps[:],
                                         cmt_t[(ei, kp)][:, :, mt * P:(mt + 1) * P],
                                         y_tiles[ei][:, kp, :, :],
                                         start=(idx == 0), stop=(idx == 7),
                                         perf_mode=DR)
                        idx += 1
                ot = outp.tile([P, O], F32, tag="out")
                nc.vector.tensor_copy(ot[:], ps[:])
                nc.sync.dma_start(pout[mt * P:(mt + 1) * P, :], ot[:])

    nc.compile()
    return nc


def get_nc():
    global _NC
    if _NC is None:
        _NC = _build()
    return _NC


def make_in_maps(x, dispatch_mask, combine_array, w1, b1, w2):
    f8 = ml_dtypes.float8_e4m3
    in_maps = []
    corrs = []
    for m in range(8):
        b, g = m // 2, m % 2
        es = slice(2 * g, 2 * g + 2)
        # dispatch: v8 = fp8(dm - 0.5), layout [e, kp, p, i, c]
        dm_s = np.transpose(dispatch_mask[b, :, es, :], (1, 0, 2)) - 0.5
        dm_s = np.ascontiguousarray(
            dm_s.reshape(2, nKP, 2, P, C).transpose(0, 1, 3, 2, 4)).astype(f8)
        # combine: fp8(cm), transposed, layout [e, kp, p, i, t]
        cmt_s = np.transpose(combine_array[b, :, es, :], (1, 2, 0))
        cmt_s = np.ascontiguousarray(
            cmt_s.reshape(2, nCP, 2, P, T).transpose(0, 1, 3, 2, 4)).astype(f8)
        # x: fp8, DR plane layout [p, kp, i, d]
        x_s = np.ascontiguousarray(
            x[b].reshape(nKP, 2, P, D).transpose(2, 0, 1, 3)).astype(f8)
        # weights: fp8(16*w), layouts [p, e, kp, i, j]
        w1_s = np.ascontiguousarray(
            (w1[es] * WS).reshape(2, nKDP, 2, P, HE)
            .transpose(3, 0, 1, 2, 4)).astype(f8)
        w2_s = np.ascontiguousarray(
            (w2[es] * WS).reshape(2, nKHP, 2, P, O)
            .transpose(3, 0, 1, 2, 4)).astype(f8)
        # GELU bias: b1 + the exact dispatch-mean term 0.5 * w1^T colsum(x)
        colsum = x[b].astype(np.float64).sum(0)
        bias = (b1[es].astype(np.float64)
                + 0.5 * np.einsum('edh,d->eh', w1[es].astype(np.float64), colsum))
        bias_s = np.ascontiguousarray(
            bias.reshape(2, nMH, P).transpose(2, 0, 1).reshape(P, 2 * nMH)
        ).astype(np.float32)
        # host-side w2 rank-1 correction ingredients
        dw2n = (w2_s.astype(np.float32)
                .transpose(1, 2, 3, 0, 4).reshape(2, HE, O) / WS
                - w2[es])                                    # [2, HE, O]
        rowsum = cmt_s.astype(np.float32).transpose(0, 1, 3, 2, 4) \
            .reshape(2, C, T).sum(1)                         # [2, T] exact from cm8
        corrs.append((dw2n, rowsum))
        in_maps.append({
            "xb": x_s, "dm": dm_s, "cmt": cmt_s,
            "w1": w1_s, "w2": w2_s, "b1s": bias_s,
        })
    return in_maps, corrs


def kernel(x, dispatch_mask, combine_array, w1, b1, w2, b2):
    nc = get_nc()
    x, dispatch_mask, combine_array, w1, b1, w2 = (
        np.asarray(a, dtype=np.float32)
        for a in (x, dispatch_mask, combine_array, w1, b1, w2))
    in_maps, corrs = make_in_maps(x, dispatch_mask, combine_array, w1, b1, w2)
    res = bass_utils.run_bass_kernel_spmd(nc, in_maps, core_ids=list(range(8)))
    b2f = np.asarray(b2, dtype=np.float32)
    out = np.empty((B, T, O), dtype=np.float32)
    for b in range(B):
        out[b] = res.results[2 * b]["pout"] + res.results[2 * b + 1]["pout"] + b2f
        for m in (2 * b, 2 * b + 1):
            dw2n, rowsum = corrs[m]
            g_acc = res.results[m]["ghs"]                    # [P, 2, 2, nMH]
            for ei in range(2):
                # ghsum[h]: h = mh*128 + p, summed over both ncc halves
                ghsum = g_acc[:, ei].sum(1).T.reshape(HE)
                M = (ghsum @ dw2n[ei]) / C                   # [O]
                out[b] -= np.outer(rowsum[ei], M)
    return out
